# revision 6
# baseline (speedup 1.0000x reference)
"""Bilinear (outer-product) pooling + signed-sqrt + L2-norm + skinny classifier.

Reference computes, for feat [B, D], W [C, D*D], b [C]:
    x[b, i*D+j] = feat[b,i] * feat[b,j]
    y = sign(x) * sqrt(|x| + EPS_SQRT)
    out = (y / max(||y||_2, EPS_NORM)) @ W.T + b

Identities (exact up to the EPS_SQRT inside the element sqrt, whose effect
on the output is ~1e-5 relative):
    y[b, i*D+j] ~= g[b,i] * g[b,j],   g = sign(feat) * sqrt(|feat|)
    ||y||_2^2    = (sum_i |feat[b,i]|)^2 + EPS_SQRT * D^2          (exact)
so with M_c = W[c].reshape(D, D):
    out[b,c] = g_b^T M_c g_b / norm_b + bias_c

Since only the symmetric part of M_c matters, stream just the upper
triangle of A_c = M_c + M_c^T in 128x128 blocks (diag blocks: M_c as-is)
-> 136 blocks = 17 per core across 8 cores, 0.53x the W traffic, cast to
bf16 on host (memory-bound problem; measured output rel err ~3e-3).

Per core, per class c (SPMD-uniform; all core variation is in the packed
data, not the program):
    slot s (one W-stationary matmul, FWL):
        ps[j, s*32+b] = sum_i A_s[i,j] * g[b, 128*bi_s + i]
    DVE:  V = ps * g_bj   (bf16)
    ones-matmul partition-reduce: ps2[0, (s,b)] = sum_j V[j, s, b]
    ACT copies ps2 into an output row buffer.
Host: out[b,c] = (sum_cores sum_slots ps2) / norm_b + bias_c.
"""

import sys

import numpy as np

if "/opt/trn_rl_repo" not in sys.path:
    sys.path.insert(0, "/opt/trn_rl_repo")

import ml_dtypes

import concourse.bass as bass
import concourse.bacc as bacc
import concourse.mybir as mybir
import concourse.tile as tile
from concourse.bass_utils import run_bass_kernel_spmd

B, D, C = 32, 2048, 30
EPS_SQRT = 1e-10
EPS_NORM = 1e-12

N_CORES = 8
P = 128
NB = D // P                              # 16 row/col blocks
NS = (NB * (NB + 1) // 2) // N_CORES     # 17 slots per core
UPPER = [(bi, bj) for bi in range(NB) for bj in range(bi, NB)]
assert len(UPPER) == NS * N_CORES
CPAIR = C // 2                           # W DMAs batched 2 classes at a time

_CACHE = {}


def _build_bass(repeat=1):
    nc = bacc.Bacc(None, target_bir_lowering=False, debug=False)
    w_d = nc.dram_tensor("w", [CPAIR, P, 2 * NS * P], mybir.dt.bfloat16,
                         kind="ExternalInput")
    gt_d = nc.dram_tensor("gt", [P, NS * B], mybir.dt.bfloat16, kind="ExternalInput")
    gc_d = nc.dram_tensor("gc", [P, NS * B], mybir.dt.float32, kind="ExternalInput")
    out_d = nc.dram_tensor("out", [1, C * NS * B], mybir.dt.float32,
                           kind="ExternalOutput")

    with tile.TileContext(nc) as tc:
        with (
            tc.tile_pool(name="wpool", bufs=5) as wpool,
            tc.tile_pool(name="const", bufs=1) as cpool,
            tc.tile_pool(name="spool", bufs=3) as spool,
            tc.tile_pool(name="psA", bufs=2, space=bass.MemorySpace.PSUM) as ppoolA,
            tc.tile_pool(name="psB", bufs=2, space=bass.MemorySpace.PSUM) as ppoolB,
        ):
            # consts ride the ACT HWDGE queue so they overlap the first W
            # transfer on the sync queue
            gt_sb = cpool.tile([P, NS * B], mybir.dt.bfloat16)
            nc.scalar.dma_start(gt_sb[:], gt_d[:])
            gc_sb = cpool.tile([P, NS * B], mybir.dt.float32)
            nc.scalar.dma_start(gc_sb[:], gc_d[:])
            ones_sb = cpool.tile([P, 1], mybir.dt.bfloat16)
            nc.vector.memset(ones_sb[:], 1.0)
            obuf = cpool.tile([1, C * NS * B], mybir.dt.float32)

            first = True
            for _ in range(repeat):
                for cp in range(CPAIR):
                    wt = wpool.tile([P, 2 * NS * P], mybir.dt.bfloat16)
                    if first:
                        # split the very first transfer so the PE starts
                        # after half a pair instead of a full one
                        nc.sync.dma_start(wt[:, :NS * P], w_d[cp, :, :NS * P])
                        nc.sync.dma_start(wt[:, NS * P:], w_d[cp, :, NS * P:])
                        first = False
                    else:
                        nc.sync.dma_start(wt[:], w_d[cp])
                    for h in range(2):
                        c = 2 * cp + h
                        wh = wt[:, h * NS * P:(h + 1) * NS * P]
                        ps = ppoolA.tile([P, NS * B], mybir.dt.float32)
                        for s in range(NS):
                            nc.tensor.matmul(
                                ps[:, s * B:(s + 1) * B],
                                wh[:, s * P:(s + 1) * P],
                                gt_sb[:, s * B:(s + 1) * B],
                                start=True, stop=True,
                            )
                        v = spool.tile([P, NS * B], mybir.dt.bfloat16)
                        nc.vector.tensor_mul(v[:], ps[:], gc_sb[:])
                        ps2 = ppoolB.tile([1, NS * B], mybir.dt.float32)
                        nc.tensor.matmul(ps2[:, 0:512], ones_sb[:], v[:, 0:512],
                                         start=True, stop=True)
                        nc.tensor.matmul(ps2[:, 512:NS * B], ones_sb[:],
                                         v[:, 512:NS * B], start=True, stop=True)
                        nc.scalar.copy(obuf[:, c * NS * B:(c + 1) * NS * B], ps2[:])
            nc.sync.dma_start(out_d[:], obuf[:])
    if not nc.is_finalized():
        nc.finalize()
    return nc


def _prep_inputs(feat, W):
    feat = np.asarray(feat, dtype=np.float32)
    W = np.asarray(W, dtype=np.float32)

    g = np.sign(feat) * np.sqrt(np.abs(feat))
    norm = np.sqrt(np.sum(np.abs(feat), axis=1, dtype=np.float64) ** 2
                   + EPS_SQRT * float(D) * float(D))
    norm = np.maximum(norm, EPS_NORM)

    W4 = W.reshape(C, NB, P, NB, P)  # [c, bi, i, bj, j]
    gT = np.ascontiguousarray(g.T)   # [D, B] fp32

    in_maps = []
    for k in range(N_CORES):
        blocks = UPPER[k::N_CORES]
        # wk[c, i, s, j] = A_c[bi_s, bj_s][i, j]
        wk = np.empty((C, P, NS, P), dtype=np.float32)
        for s, (bi, bj) in enumerate(blocks):
            blk = W4[:, bi, :, bj, :]
            if bi != bj:
                blk = blk + W4[:, bj, :, bi, :].transpose(0, 2, 1)
            wk[:, :, s, :] = blk
        wk = (wk.astype(ml_dtypes.bfloat16)
                .reshape(CPAIR, 2, P, NS * P)
                .transpose(0, 2, 1, 3))          # [cpair, i, half, s*j]
        wk = np.ascontiguousarray(wk).reshape(CPAIR, P, 2 * NS * P)
        gt = np.empty((P, NS, B), dtype=np.float32)
        gc = np.empty((P, NS, B), dtype=np.float32)
        for s, (bi, bj) in enumerate(blocks):
            gt[:, s, :] = gT[bi * P:(bi + 1) * P, :]
            gc[:, s, :] = gT[bj * P:(bj + 1) * P, :]
        in_maps.append({
            "w": wk,
            "gt": gt.reshape(P, NS * B).astype(ml_dtypes.bfloat16),
            "gc": np.ascontiguousarray(gc.reshape(P, NS * B)),
        })
    return in_maps, norm


def _run(inputs, trace=False, repeat=1):
    feat, W, b = inputs["feat"], inputs["W"], inputs["b"]
    assert feat.shape == (B, D) and W.shape == (C, D * D)

    key = ("nc", repeat)
    if key not in _CACHE:
        _CACHE[key] = _build_bass(repeat)
    nc = _CACHE[key]

    in_maps, norm = _prep_inputs(feat, W)
    res = run_bass_kernel_spmd(nc, in_maps, list(range(N_CORES)), trace=trace)
    parts = np.stack([r["out"] for r in res.results]).astype(np.float64)
    parts = parts.reshape(N_CORES, C, NS, B).sum(axis=(0, 2)).T  # [B, C]
    out = parts / norm[:, None] + np.asarray(b, dtype=np.float64)[None, :]
    return out.astype(np.float32), res


def kernel(**inputs):
    return _run(inputs)[0]
